# revision 32
# baseline (speedup 1.0000x reference)
"""Trainium2 Bass kernel for nn_CRANModel (CRAN-style memory recurrence).

Strategy
--------
Cache *keys* depend only on token embeddings, so scores/top-8/softmax are
precomputed in one batched phase.  The serial *value* path reduces to

    h_t = tanh(U'_t + (A_t @ Hbar[:t]) @ C'),   Hbar_j = (1/B) 1^T h_j

with U' = [X|R] @ Wh + bh - A@D0 (batched), A = masked top-8 weights on the
first 64 slots, C' = Wv @ Wh_r / B.

The 64-step scan runs TRANSPOSED: pre-activations accumulate as
pz^T [H-chunk x 128, 4 steps * 32 batch] PSUM blocks.  Per step the PE does
  inject U' (4x32) -> y = A@Hbar (4x32) -> z = y@C' (16x32, bf16 blocks)
  -> Hbar-table rebuild via one-hot wideH @ ident (4x128)
~1280 cycles/step vs 2304 for the direct G-table form.

The 131 MB bf16 logits projection (the memory roofline) is sharded over
vocab across the 8 cores, computed in bf16 (same PE rate, half the
SBUF/DMA), and interleaved into the scan as PE filler; Wout is prefetched
during the DMA-idle phase-0 window; the final steps use narrow column
chunks so only the last step's 32 columns remain after the last tanh.
"""

import sys
import numpy as np
import ml_dtypes

for p in ("/opt/trn_rl_repo", "/root/.axon_site/_ro/trn_rl_repo"):
    if p not in sys.path:
        sys.path.append(p)

# problem dims (hardcoded per contract)
T, B, V, E, H, N, DK, DV = 64, 32, 32000, 512, 512, 512, 256, 512
K = 8
NCORES = 8
TB = T * B                   # 2048 rows
RG = TB // 128               # 16 row groups of 128
VSH = V // NCORES            # 4000 vocab columns per core
VCH = (VSH + 127) // 128     # 32 v-chunks per core (last is ragged: 32 rows)
VLAST = VSH - (VCH - 1) * 128
# projection column chunks: (col0, width); chunk ready after step
# (col0+width)/B - 1 completes. Coarse 256-col chunks early, narrow at the
# end so only the final 32 columns trail the last tanh.
PROJ_CHUNKS = [(c * 256, 256) for c in range(7)] + \
    [(1792, 128), (1920, 64), (1984, 32), (2016, 32)]
_SCORES_F32R = True          # score path in f32r (1cyc/row); fp32 fallback
_REPEAT = 1
_DEBUG = False
_NOZ = False
_ZERO_Y = False


def _round_f32r(a):
    """Round-to-nearest-even to 11 explicit mantissa bits (fp32r)."""
    u = np.ascontiguousarray(a, np.float32).view(np.uint32)
    u = (u + 0x7FF + ((u >> 12) & 1)) & np.uint32(0xFFFFF000)
    return u.view(np.float32)


def _build_program(repeat=1):
    import contextlib
    import concourse.bass as bass
    import concourse.mybir as mybir
    import concourse.tile as tile
    from concourse import bacc
    from concourse.masks import make_identity

    f32 = mybir.dt.float32
    f32r = mybir.dt.float32r
    bf16 = mybir.dt.bfloat16
    ACT = mybir.ActivationFunctionType

    nc = bacc.Bacc("TRN2", debug=False, target_bir_lowering=False)

    # ---------------- DRAM I/O ----------------
    d_tok = nc.dram_tensor("tok", [128, RG], mybir.dt.int32, kind="ExternalInput").ap()
    d_emb = nc.dram_tensor("emb", [V, E], f32, kind="ExternalInput").ap()
    fsc = f32r if _SCORES_F32R else f32
    d_wq = nc.dram_tensor("wq", [E, DK], fsc, kind="ExternalInput").ap()
    d_wk = nc.dram_tensor("wk", [E, DK], fsc, kind="ExternalInput").ap()
    d_k0T = nc.dram_tensor("k0T", [DK, N], fsc, kind="ExternalInput").ap()
    d_wh = nc.dram_tensor("wh", [E + DV, H], f32r, kind="ExternalInput").ap()
    d_wvT = nc.dram_tensor("wvT", [DV, H], f32r, kind="ExternalInput").ap()
    d_v0 = nc.dram_tensor("v0", [N, DV], bf16, kind="ExternalInput").ap()
    d_v0hT = nc.dram_tensor("v0hT", [DV, T], f32r, kind="ExternalInput").ap()
    d_bhb = nc.dram_tensor("bhb", [1, H], f32, kind="ExternalInput").ap()
    d_maskRM = nc.dram_tensor("maskRM", [128, RG, T], f32, kind="ExternalInput").ap()
    d_wout = nc.dram_tensor("woutc", [128, 4, VCH * 128], bf16,
                            kind="ExternalInput").ap()
    d_boutT = nc.dram_tensor("boutc", [128, VCH], f32, kind="ExternalInput").ap()
    d_boutR = nc.dram_tensor("boutr", [1, VCH * 128], bf16,
                             kind="ExternalInput").ap()
    d_out = nc.dram_tensor("out", [VSH, TB], bf16, kind="ExternalOutput").ap()
    if _DEBUG:
        d_dbgU = nc.dram_tensor("dbgU", [128, RG, H], f32,
                                kind="ExternalOutput").ap()
        d_dbgAT = nc.dram_tensor("dbgAT", [T, TB], f32,
                                 kind="ExternalOutput").ap()
        d_dbgHb = nc.dram_tensor("dbgHb", [T, H], f32,
                                 kind="ExternalOutput").ap()
        d_dbgHT = nc.dram_tensor("dbgHT", [128, 4, TB], mybir.dt.uint16,
                                 kind="ExternalOutput").ap()
        d_dbgY = nc.dram_tensor("dbgY", [128, 4, 8 * B], f32,
                                kind="ExternalOutput").ap()

    with tile.TileContext(nc) as tc:
        with contextlib.ExitStack() as stack:
            cst = stack.enter_context(tc.tile_pool(name="cst", bufs=1))

            ident = cst.tile([128, 128], f32)
            make_identity(nc, ident)
            ident_bf = cst.tile([128, 128], bf16)
            nc.vector.tensor_copy(out=ident_bf[:], in_=ident[:])
            tok_sb = cst.tile([128, RG], mybir.dt.int32)
            nc.sync.dma_start(tok_sb[:], d_tok[:])
            boutT_sb = cst.tile([128, VCH], f32)
            nc.sync.dma_start(boutT_sb[:], d_boutT[:])
            ones_c = cst.tile([1, 128], f32)
            nc.vector.memset(ones_c[:], 1.0)
            ones_bf = cst.tile([1, 128], bf16)
            nc.vector.tensor_copy(out=ones_bf[:], in_=ones_c[:])

            # persistent tensors for the scan + projection
            big = stack.enter_context(tc.tile_pool(name="big", bufs=1))
            AT_bf = big.tile([T, TB], bf16)       # masked A^T  [slot, row]
            U_bf = big.tile([128, RG, H], bf16)   # U' rows per 4-step group
            c2_bf = big.tile([128, 4, H], bf16)   # C' = Wv @ Wh_r / B
            Hb_bf = big.tile([T, H], bf16)        # Hbar rows (batch-mean h)
            wideH = big.tile([128, 4, T], bf16)   # hbar^T columns, col t
            hT_sb = big.tile([128, 4, TB], bf16)  # h^T for the projection
            wout_a = big.tile([128, 4, VCH * 64], bf16)
            wout_b = big.tile([128, 4, VCH * 64], bf16)
            boutR_bf = big.tile([1, VCH * 128], bf16)  # bias row, PE bias-add
            nc.sync.dma_start(boutR_bf[:], d_boutR[:])

            for _rep in range(repeat):
                scan_pools = {}

                def sc_pool(name):
                    return scan_pools[name]

                def scan_step(t, fillers=()):
                    j, b = t & 3, t // 4
                    fillers = list(fillers)
                    # per-step psum tile. NOTE: PSUM tracks ONE open
                    # accumulation group per bank, so each region m's group
                    # (U' inject + z) must open and close before the next.
                    pzt = sc_pool("ps_blk").tile([128, 4, B], f32, tag="pzt",
                                                 name="pzt")
                    if t == 0:
                        for m in range(4):
                            msl = slice(m * 128, (m + 1) * 128)
                            nc.tensor.matmul(
                                out=pzt[:, m, :], lhsT=U_bf[:, b, msl],
                                rhs=ident_bf[:, j * B:(j + 1) * B],
                                start=True, stop=True)
                    else:
                        # y^T = Hbar[:t]^T A_t^T  [dv-chunk x 128, B]
                        py_ = sc_pool("ps_y").tile([128, 4, B], f32,
                                                   tag="py", name="py")
                        for d in range(4):
                            nc.tensor.matmul(
                                out=py_[:, d, :],
                                lhsT=Hb_bf[0:t, d * 128:(d + 1) * 128],
                                rhs=AT_bf[0:t, t * B:(t + 1) * B],
                                start=True, stop=True)
                        y_sb = sc_pool("sc").tile([128, 4, B], bf16,
                                                  tag="ysb", name="ysb")
                        nc.vector.tensor_copy(out=y_sb[:], in_=py_[:])
                        for f in fillers[:2]:
                            f()
                        # region m: U' inject (start) + z = y @ C' (stop)
                        for m in range(4):
                            msl = slice(m * 128, (m + 1) * 128)
                            nc.tensor.matmul(
                                out=pzt[:, m, :], lhsT=U_bf[:, b, msl],
                                rhs=ident_bf[:, j * B:(j + 1) * B],
                                start=True, stop=False)
                            for d in range(4):
                                nc.tensor.matmul(
                                    out=pzt[:, m, :],
                                    lhsT=c2_bf[:, d, msl],
                                    rhs=y_sb[:, d, :],
                                    start=False, stop=(d == 3))

                    # tanh -> h^T columns (bf16), one instruction
                    nc.scalar.activation(
                        out=hT_sb[:, :, t * B:(t + 1) * B],
                        in_=pzt[:],
                        func=ACT.Tanh)

                    # hbar^T = batch-sum of h^T -> column t of wideH
                    with nc.allow_low_precision(
                            reason="hbar rounded to bf16 for the PE"):
                        nc.vector.reduce_sum(
                            out=wideH[:, :, t:t + 1],
                            in_=hT_sb[:, :, t * B:(t + 1) * B],
                            axis=mybir.AxisListType.X)

                    for f in fillers[2:4]:
                        f()

                    # rebuild Hbar table rows 0..t from the accumulated
                    # one-hot columns (cols > t are still zero); only row t
                    # is new -- mirror it to SBUF for the next step's y.
                    psum_Hb = sc_pool("ps_hb").tile([T, 4, 128], f32,
                                                    tag="phb", name="phb")
                    for k in range(4):
                        nc.tensor.matmul(
                            out=psum_Hb[:, k, :], lhsT=wideH[:, k, :],
                            rhs=ident_bf[:, 0:128],
                            start=True, stop=True)
                    blk = (t // 32) * 32
                    nc.scalar.copy(
                        out=Hb_bf[blk:blk + 32, 0:256].rearrange(
                            "p (a c) -> p a c", c=128),
                        in_=psum_Hb[blk:blk + 32, 0:2, :])
                    nc.vector.tensor_copy(
                        out=Hb_bf[blk:blk + 32, 256:512].rearrange(
                            "p (a c) -> p a c", c=128),
                        in_=psum_Hb[blk:blk + 32, 2:4, :])

                    for f in fillers[4:]:
                        f()

                # =================== PHASE 0 ===================
                with contextlib.ExitStack() as ph0:
                    w0 = ph0.enter_context(tc.tile_pool(name="w0", bufs=1))
                    xt_p = ph0.enter_context(tc.tile_pool(name="xt", bufs=1))
                    p0 = ph0.enter_context(tc.tile_pool(name="p0", bufs=1))
                    sg_p = ph0.enter_context(
                        tc.tile_pool(name="sg", bufs=2))
                    pp = ph0.enter_context(tc.tile_pool(name="pp", bufs=2))
                    px = ph0.enter_context(tc.tile_pool(name="px", bufs=3))
                    wp = ph0.enter_context(tc.tile_pool(name="wp", bufs=2))
                    qp = ph0.enter_context(tc.tile_pool(name="qp", bufs=1))
                    ps_mm = ph0.enter_context(
                        tc.tile_pool(name="ps_mm", bufs=4, space="PSUM"))
                    ps_tr = ph0.enter_context(
                        tc.tile_pool(name="ps_tr", bufs=2, space="PSUM"))

                    wq_sb = w0.tile([128, 4, DK], fsc)
                    nc.sync.dma_start(
                        wq_sb[:], d_wq.rearrange("(c p) m -> p c m", p=128))
                    wk_sb = w0.tile([128, 4, DK], fsc)
                    nc.sync.dma_start(
                        wk_sb[:], d_wk.rearrange("(c p) m -> p c m", p=128))
                    k0T_sb = w0.tile([128, 2, N], fsc)
                    nc.sync.dma_start(
                        k0T_sb[:], d_k0T.rearrange("(c p) m -> p c m", p=128))
                    # chunked loads: keep individual transfers ~0.7 us so
                    # the pass-A gather DMAs are never stuck behind them
                    wh_sb = w0.tile([128, 8, H], f32r)
                    whr_ap = d_wh.rearrange("(c p) m -> p c m", p=128)
                    for c8 in range(8):
                        nc.sync.dma_start(wh_sb[:, c8, :], whr_ap[:, c8, :])
                    wvT_sb = w0.tile([128, 4, H], f32r)
                    wvr_ap = d_wvT.rearrange("(c p) m -> p c m", p=128)
                    for c4 in range(4):
                        nc.sync.dma_start(wvT_sb[:, c4, :], wvr_ap[:, c4, :])
                    v0_sb = w0.tile([128, 4, DV], bf16)
                    v0r_ap = d_v0.rearrange("(c p) m -> p c m", p=128)
                    for c4 in range(4):
                        nc.sync.dma_start(v0_sb[:, c4, :], v0r_ap[:, c4, :])
                    v0hT_sb = w0.tile([128, 4, T], f32r)
                    nc.sync.dma_start(
                        v0hT_sb[:], d_v0hT.rearrange("(c p) m -> p c m", p=128))
                    bhb_sb = w0.tile([1, H], f32)
                    nc.sync.dma_start(bhb_sb[:], d_bhb[:])
                    bhr_sb = w0.tile([1, H], f32r)
                    nc.vector.tensor_copy(out=bhr_sb[:], in_=bhb_sb[:])
                    ones32 = w0.tile([1, 128], f32)
                    nc.vector.memset(ones32[:], 1.0)
                    onesr = w0.tile([1, 128], f32r)
                    nc.vector.tensor_copy(out=onesr[:], in_=ones32[:])
                    maskRM_sb = w0.tile([128, RG, T], f32)
                    nc.sync.dma_start(maskRM_sb[:], d_maskRM[:])
                    negD0 = w0.tile([T, H], bf16)
                    # Wout prefetch: issued after every phase-0 input in SP
                    # program order, so the transfers fill the DMA-idle
                    # pass-B window instead of competing with the gathers.
                    wouta_ap = d_wout[:, :, 0:VCH * 64].rearrange(
                        "p q (c m) -> p q c m", c=4)
                    woutb_ap = d_wout[:, :, VCH * 64:].rearrange(
                        "p q (c m) -> p q c m", c=4)
                    wa_t = wout_a.rearrange("p q (c m) -> p q c m", c=4)
                    wb_t = wout_b.rearrange("p q (c m) -> p q c m", c=4)
                    for q in range(4):
                        for c in range(4):
                            nc.sync.dma_start(wa_t[:, q, c, :],
                                              wouta_ap[:, q, c, :])
                    for q in range(4):
                        for c in range(4):
                            nc.sync.dma_start(wb_t[:, q, c, :],
                                              woutb_ap[:, q, c, :])
                    nc.vector.memset(wideH[:].bitcast(f32), 0.0)

                    xT_sb = xt_p.tile([128, 4, TB], fsc)
                    xbT_sb = xt_p.tile([128, 4, T], fsc)
                    knT_sb = xt_p.tile([128, 2, T], fsc)

                    # --- pass A: gather X = emb[tok], transpose into xT ---
                    for g in range(RG):
                        xg = px.tile([128, E], f32, tag="xg")
                        nc.gpsimd.indirect_dma_start(
                            out=xg[:], out_offset=None, in_=d_emb[:],
                            in_offset=bass.IndirectOffsetOnAxis(
                                ap=tok_sb[:, g:g + 1], axis=0),
                        )
                        for e in range(4):
                            ptr = ps_tr.tile([128, 128], f32, tag="ptr")
                            nc.tensor.transpose(
                                out=ptr[:], in_=xg[:, e * 128:(e + 1) * 128],
                                identity=ident[:])
                            if e % 2 == 0:
                                nc.scalar.copy(
                                    out=xT_sb[:, e, g * 128:(g + 1) * 128],
                                    in_=ptr[:])
                            else:
                                nc.vector.tensor_copy(
                                    out=xT_sb[:, e, g * 128:(g + 1) * 128],
                                    in_=ptr[:])

                    # --- Xbar^T (batch sums; 1/B folded into Knew evict) ---
                    with nc.allow_low_precision(
                            reason="batch-mean rounded to f32r for the PE; "
                                   "accumulator is fp32"):
                        for e in range(4):
                            nc.vector.reduce_sum(
                                out=xbT_sb[:, e, :],
                                in_=xT_sb[:, e, :].rearrange(
                                    "p (t b) -> p t b", b=B),
                                axis=mybir.AxisListType.X)

                    # --- Knew^T = Wk^T Xbar^T / B ---
                    for m2 in range(2):
                        pk = ps_mm.tile([128, 512], f32, tag="pmm")
                        for e in range(4):
                            nc.tensor.matmul(
                                out=pk[:, 0:T],
                                lhsT=wk_sb[:, e, m2 * 128:(m2 + 1) * 128],
                                rhs=xbT_sb[:, e, :],
                                start=(e == 0), stop=(e == 3))
                        nc.scalar.activation(
                            out=knT_sb[:, m2, :], in_=pk[:, 0:T],
                            func=ACT.Copy, scale=float(1.0 / B))

                    # --- C' = Wv @ Wh_r / B ;  negD0 = -values0[:64] @ Wh_r ---
                    for m4 in range(4):
                        pc = ps_mm.tile([128, H], f32, tag="pmm")
                        for d4 in range(4):
                            nc.tensor.matmul(
                                out=pc[:],
                                lhsT=wvT_sb[:, d4, m4 * 128:(m4 + 1) * 128],
                                rhs=wh_sb[:, 4 + d4, :], start=(d4 == 0),
                                stop=(d4 == 3))
                        with nc.allow_low_precision(
                                reason="C' rounded to bf16 for the PE"):
                            nc.scalar.activation(
                                out=c2_bf[:, m4, :], in_=pc[:],
                                func=ACT.Copy, scale=float(1.0 / B))
                    pd = ps_mm.tile([128, H], f32, tag="pmm")
                    for d4 in range(4):
                        nc.tensor.matmul(
                            out=pd[0:T, :], lhsT=v0hT_sb[:, d4, :],
                            rhs=wh_sb[:, 4 + d4, :], start=(d4 == 0),
                            stop=(d4 == 3))
                    nc.scalar.activation(out=negD0[:], in_=pd[0:T, :],
                                         func=ACT.Copy, scale=-1.0)


                    # --- pass B: per quad of row-groups (N=512 matmuls),
                    # software-pipelined: scores/top-8 of q4 run while the
                    # transposes/R/U of q4-1 occupy the PE.
                    def emit_front(q4):
                        qsl = slice(q4 * 512, (q4 + 1) * 512)

                        qT4 = pp.tile([128, 2, 512], fsc, tag="qT4")
                        for m2 in range(2):
                            pq = ps_mm.tile([128, 512], f32, tag="pmm")
                            for e in range(4):
                                nc.tensor.matmul(
                                    out=pq[:],
                                    lhsT=wq_sb[:, e, m2 * 128:(m2 + 1) * 128],
                                    rhs=xT_sb[:, e, qsl],
                                    start=(e == 0), stop=(e == 3))
                            nc.scalar.activation(
                                out=qT4[:, m2, :], in_=pq[:],
                                func=ACT.Copy, scale=float(1.0 / np.sqrt(DK)))

                        wgs = []
                        for gl in range(4):
                            g = q4 * 4 + gl
                            lsl = slice(gl * 128, (gl + 1) * 128)

                            s_g = sg_p.tile([128, N], f32, tag="sg")
                            ps_s = ps_mm.tile([128, N], f32, tag="pmm")
                            for k2 in range(2):
                                nc.tensor.matmul(
                                    out=ps_s[:], lhsT=qT4[:, k2, lsl],
                                    rhs=k0T_sb[:, k2, :],
                                    start=(k2 == 0), stop=(k2 == 1))
                            nc.scalar.copy(out=s_g[:], in_=ps_s[:])
                            ps_n = ps_mm.tile([128, N], f32, tag="pmm")
                            for k2 in range(2):
                                nc.tensor.matmul(
                                    out=ps_n[:, 0:T], lhsT=qT4[:, k2, lsl],
                                    rhs=knT_sb[:, k2, :],
                                    start=(k2 == 0), stop=(k2 == 1))
                            nc.vector.copy_predicated(
                                out=s_g[:, 0:T],
                                mask=maskRM_sb[:, g, :].bitcast(mybir.dt.uint32),
                                data=ps_n[:, 0:T])

                            # top-8 softmax, normalizer folded into exp bias:
                            # w = (s >= thr) * exp(s - mx - ln z)
                            mx = p0.tile([128, 8], f32, tag=f"mx{gl}")
                            nc.vector.max(out=mx[:], in_=s_g[:])
                            negm1 = p0.tile([128, 1], f32, tag=f"nm{gl}")
                            nc.vector.tensor_scalar_mul(negm1[:], mx[:, 0:1],
                                                        -1.0)
                            emx = p0.tile([128, 8], f32, tag=f"em{gl}")
                            nc.scalar.activation(out=emx[:], in_=mx[:],
                                                 func=ACT.Exp,
                                                 bias=negm1[:, 0:1])
                            zrow = p0.tile([128, 1], f32, tag=f"zr{gl}")
                            nc.vector.reduce_sum(out=zrow[:], in_=emx[:],
                                                 axis=mybir.AxisListType.X)
                            winv = p0.tile([128, 1], f32, tag=f"wi{gl}")
                            nc.vector.reciprocal(out=winv[:], in_=zrow[:])
                            w_g = wp.tile([128, N], bf16, tag=f"wg{gl}")
                            with nc.allow_low_precision(
                                    reason="softmax weights to bf16 for PE"):
                                nc.scalar.activation(out=w_g[:], in_=s_g[:],
                                                     func=ACT.Exp,
                                                     bias=negm1[:, 0:1])
                                nc.vector.scalar_tensor_tensor(
                                    out=w_g[:], in0=s_g[:], scalar=mx[:, 7:8],
                                    in1=w_g[:], op0=mybir.AluOpType.is_ge,
                                    op1=mybir.AluOpType.mult)
                                nc.vector.tensor_scalar_mul(w_g[:], w_g[:],
                                                            winv[:, 0:1])
                            am = wp.tile([128, T], bf16, tag=f"am{gl}")
                            nc.gpsimd.tensor_mul(am[:], w_g[:, 0:T],
                                                 maskRM_sb[:, g, :])
                            wgs.append((w_g, am))
                        return wgs

                    def emit_back(q4, wgs):
                        # transposes into A^T / Wfull^T
                        wfT4 = qp.tile([128, 4, 512], bf16, tag="wfT4")
                        for gl in range(4):
                            g = q4 * 4 + gl
                            gsl = slice(g * 128, (g + 1) * 128)
                            lsl = slice(gl * 128, (gl + 1) * 128)
                            w_g, am = wgs[gl]

                            pat = ps_tr.tile([128, 128], bf16, tag="ptrb")
                            nc.tensor.transpose(out=pat[0:T, :], in_=am[:],
                                                identity=ident_bf[:])
                            nc.vector.tensor_copy(out=AT_bf[:, gsl],
                                                  in_=pat[0:T, :])

                            for s4 in range(4):
                                ptr = ps_tr.tile([128, 128], bf16, tag="ptrb")
                                nc.tensor.transpose(
                                    out=ptr[:],
                                    in_=w_g[:, s4 * 128:(s4 + 1) * 128],
                                    identity=ident_bf[:])
                                with nc.allow_low_precision(
                                        reason="w^T to bf16 for the PE"):
                                    if s4 % 2 == 0:
                                        nc.scalar.copy(out=wfT4[:, s4, lsl],
                                                       in_=ptr[:])
                                    else:
                                        nc.vector.tensor_copy(
                                            out=wfT4[:, s4, lsl], in_=ptr[:])

                        # R^T quad = values0^T @ Wfull^T   (f32r, N=512)
                        rT4 = qp.tile([128, 4, 512], f32r, tag="rT4")
                        for m4 in range(4):
                            pr = ps_mm.tile([128, 512], f32, tag="pmm")
                            for s4 in range(4):
                                nc.tensor.matmul(
                                    out=pr[:],
                                    lhsT=v0_sb[:, s4, m4 * 128:(m4 + 1) * 128],
                                    rhs=wfT4[:, s4, :],
                                    start=(s4 == 0), stop=(s4 == 3))
                            if m4 % 2 == 0:
                                nc.vector.tensor_copy(out=rT4[:, m4, :],
                                                      in_=pr[:])
                            else:
                                nc.scalar.copy(out=rT4[:, m4, :], in_=pr[:])

                        # U' rows = [X|R] @ Wh + bh + A@(-D0)   -> bf16
                        for gl in range(4):
                            g = q4 * 4 + gl
                            gsl = slice(g * 128, (g + 1) * 128)
                            lsl = slice(gl * 128, (gl + 1) * 128)
                            pu = ps_mm.tile([128, H], f32, tag="pmm")
                            for e in range(4):
                                nc.tensor.matmul(
                                    out=pu[:], lhsT=xT_sb[:, e, gsl],
                                    rhs=wh_sb[:, e, :], start=(e == 0),
                                    stop=False)
                            for d4 in range(4):
                                nc.tensor.matmul(
                                    out=pu[:], lhsT=rT4[:, d4, lsl],
                                    rhs=wh_sb[:, 4 + d4, :], start=False,
                                    stop=False)
                            nc.tensor.matmul(
                                out=pu[:], lhsT=onesr[:], rhs=bhr_sb[:],
                                start=False, stop=False)
                            nc.tensor.matmul(
                                out=pu[:], lhsT=AT_bf[:, gsl], rhs=negD0[:],
                                start=False, stop=True)
                            if gl % 2 == 0:
                                nc.vector.tensor_copy(out=U_bf[:, g, :],
                                                      in_=pu[:])
                            else:
                                nc.scalar.copy(out=U_bf[:, g, :], in_=pu[:])

                    fronts = {0: emit_front(0)}
                    for q4 in range(4):
                        if q4 + 1 < 4:
                            fronts[q4 + 1] = emit_front(q4 + 1)
                        emit_back(q4, fronts.pop(q4))

                # ===== scan with interleaved projection =====
                with contextlib.ExitStack() as ph1:
                    ob_p = ph1.enter_context(tc.tile_pool(name="ob", bufs=2))
                    if _DEBUG:
                        dbgy_p = ph1.enter_context(
                            tc.tile_pool(name="dbgy", bufs=1))
                        scan_pools["dbgY_tile"] = dbgy_p.tile(
                            [128, 4, 8 * B], f32, name="dbgY_tile")
                        scan_pools["dbgY"] = []
                    scan_pools["sc"] = ph1.enter_context(
                        tc.tile_pool(name="sc", bufs=3))
                    scan_pools["ps_blk"] = ph1.enter_context(
                        tc.tile_pool(name="ps_blk", bufs=3, space="PSUM"))
                    scan_pools["ps_y"] = ph1.enter_context(
                        tc.tile_pool(name="ps_y", bufs=2, space="PSUM"))
                    scan_pools["ps_hb"] = ph1.enter_context(
                        tc.tile_pool(name="ps_hb", bufs=1, space="PSUM"))
                    ps_o = ph1.enter_context(
                        tc.tile_pool(name="ps_o", bufs=2, space="PSUM"))

                    # projection units. Wide chunks (cw >= 128): one unit per
                    # vc, ACT/DVE bias-add epilogue. Narrow chunks (cw < 128):
                    # one unit per 4 vc with the bias added on the PE (rank-1
                    # matmul) so the epilogue is a single copy.
                    ob_tiles = {}

                    def dma_group(ci, vc, csl, cw, ob):
                        v0c = vc - 3
                        if vc < VCH - 1:
                            nc.sync.dma_start(
                                d_out[v0c * 128:(vc + 1) * 128, csl]
                                .rearrange("(v p) c -> p v c", p=128),
                                ob[:, :, 0:cw])
                        else:
                            nc.sync.dma_start(
                                d_out[v0c * 128:vc * 128, csl]
                                .rearrange("(v p) c -> p v c", p=128),
                                ob[:, 0:3, 0:cw])
                            nc.sync.dma_start(
                                d_out[vc * 128:vc * 128 + VLAST, csl],
                                ob[0:VLAST, 3, 0:cw])

                    def proj_unit_w(ci, vc):
                        col0, cw = PROJ_CHUNKS[ci]
                        vsz = 128 if vc < VCH - 1 else VLAST
                        csl = slice(col0, col0 + cw)
                        po = ps_o.tile([128, 256], f32, tag="po")
                        wsb = wout_a if vc < VCH // 2 else wout_b
                        vr = vc % (VCH // 2)
                        for hc in range(4):
                            nc.tensor.matmul(
                                out=po[:, 0:cw],
                                lhsT=wsb[:, hc, vr * 128:(vr + 1) * 128],
                                rhs=hT_sb[:, hc, csl],
                                start=(hc == 0), stop=(hc == 3))
                        if vc % 4 == 0:
                            ob_tiles[ci] = ob_p.tile([128, 4, 256], bf16,
                                                     tag="ob", name="ob")
                        ob = ob_tiles[ci]
                        if vc % 2 == 0:
                            nc.scalar.activation(
                                out=ob[0:vsz, vc % 4, 0:cw], in_=po[0:vsz, 0:cw],
                                func=ACT.Identity,
                                bias=boutT_sb[0:vsz, vc:vc + 1])
                        else:
                            nc.vector.tensor_scalar_add(
                                ob[0:vsz, vc % 4, 0:cw], po[0:vsz, 0:cw],
                                boutT_sb[0:vsz, vc:vc + 1])
                        if vc % 4 == 3:
                            dma_group(ci, vc, csl, cw, ob)

                    def proj_unit_n(ci, vc0):
                        col0, cw = PROJ_CHUNKS[ci]
                        csl = slice(col0, col0 + cw)
                        po = ps_o.tile([128, 256], f32, tag="po")
                        pv = po[:].rearrange("p (v c) -> p v c", c=64)
                        for vi in range(4):
                            vc = vc0 + vi
                            wsb = wout_a if vc < VCH // 2 else wout_b
                            vr = vc % (VCH // 2)
                            for hc in range(4):
                                nc.tensor.matmul(
                                    out=pv[:, vi, 0:cw],
                                    lhsT=wsb[:, hc, vr * 128:(vr + 1) * 128],
                                    rhs=hT_sb[:, hc, csl],
                                    start=(hc == 0), stop=False)
                            nc.tensor.matmul(
                                out=pv[:, vi, 0:cw],
                                lhsT=boutR_bf[0:1, vc * 128:(vc + 1) * 128],
                                rhs=ones_bf[0:1, 0:cw],
                                start=False, stop=True)
                        ob = ob_p.tile([128, 4, 256], bf16, tag="ob",
                                       name="ob")
                        obv = ob[:].rearrange("p v (s c) -> p v s c", c=64)
                        if (vc0 // 4) % 2 == 0:
                            nc.scalar.copy(out=obv[:, :, 0, 0:cw], in_=pv[:, :, 0:cw])
                        else:
                            nc.vector.tensor_copy(out=obv[:, :, 0, 0:cw],
                                                  in_=pv[:, :, 0:cw])
                        dma_group(ci, vc0 + 3, csl, cw, obv[:, :, 0, :])

                    # chunk ci's last column is for step (col0+cw)/B - 1, so
                    # it becomes computable at the following step. Schedule
                    # by PE-work budget per step so the serial chain always
                    # has filler without flooding any single step.
                    units = []
                    for ci, (col0, cw) in enumerate(PROJ_CHUNKS):
                        t_rdy = min((col0 + cw) // B, T - 1)
                        if cw >= 128:
                            for vc in range(VCH):
                                units.append((t_rdy, 4 * cw * 0.42,
                                              ("w", ci, vc)))
                        else:
                            for vc0 in range(0, VCH, 4):
                                units.append((t_rdy, 20 * cw * 0.42,
                                              ("n", ci, vc0)))
                    units.sort(key=lambda u: u[0])
                    proj_sched = {t: [] for t in range(T)}
                    qi, queue = 0, []
                    for t in range(T):
                        while qi < len(units) and units[qi][0] <= t:
                            queue.append(units[qi])
                            qi += 1
                        budget = 2200.0
                        while queue and (budget > 0 or t == T - 1):
                            u = queue.pop(0)
                            proj_sched[t].append(u[2])
                            budget -= u[1]

                    def mk_unit(spec):
                        kind, ci, vc = spec
                        if kind == "w":
                            return lambda: proj_unit_w(ci, vc)
                        return lambda: proj_unit_n(ci, vc)

                    for t in range(0, T):
                        scan_step(t, [mk_unit(s) for s in proj_sched[t]])

                    if _DEBUG:
                        dbg_p = ph1.enter_context(
                            tc.tile_pool(name="dbg", bufs=1))
                        dbgU = dbg_p.tile([128, RG, H], f32)
                        nc.vector.tensor_copy(out=dbgU[:], in_=U_bf[:])
                        nc.sync.dma_start(d_dbgU[:], dbgU[:])
                        dbgAT = dbg_p.tile([T, TB], f32)
                        nc.vector.tensor_copy(out=dbgAT[:], in_=AT_bf[:])
                        nc.sync.dma_start(d_dbgAT[:], dbgAT[:])
                        dbgHb = dbg_p.tile([T, H], f32)
                        nc.vector.tensor_copy(out=dbgHb[:], in_=Hb_bf[:])
                        nc.sync.dma_start(d_dbgHb[:], dbgHb[:])
                        nc.sync.dma_start(
                            d_dbgHT[:], hT_sb[:].bitcast(mybir.dt.uint16))
                        nc.sync.dma_start(d_dbgY[:],
                                          scan_pools["dbgY_tile"][:])

    nc.compile()
    return nc


_CACHE = {}


def _get_program():
    key = ("nc", _REPEAT, _DEBUG)
    if key not in _CACHE:
        _CACHE[key] = _build_program(repeat=_REPEAT)
    return _CACHE[key]


def _host_prep(tokens, emb, Wq, Wk, Wv, Wh, bh, Wout, bout, keys0, values0):
    tok = np.ascontiguousarray(
        np.asarray(tokens, np.int64).reshape(TB).astype(np.int32))
    tok_cm = np.zeros((128, RG), np.int32)
    for g in range(RG):
        tok_cm[:, g] = tok[g * 128:(g + 1) * 128]

    t_of_row = np.repeat(np.arange(T), B)                      # [TB]
    maskRM = (np.arange(T)[None, :] < t_of_row[:, None]).astype(np.float32)
    maskRM_cm = np.zeros((128, RG, T), np.float32)
    for g in range(RG):
        maskRM_cm[:, g, :] = maskRM[g * 128:(g + 1) * 128]

    fsc = _round_f32r if _SCORES_F32R else np.ascontiguousarray
    base = {
        "tok": tok_cm,
        "emb": np.ascontiguousarray(np.asarray(emb, np.float32)),
        "wq": fsc(np.asarray(Wq, np.float32)),
        "wk": fsc(np.asarray(Wk, np.float32)),
        "k0T": fsc(np.asarray(keys0, np.float32).T),
        "wh": _round_f32r(np.asarray(Wh, np.float32)),
        "wvT": _round_f32r(np.asarray(Wv, np.float32).T),
        "v0": np.asarray(values0, np.float32).astype(ml_dtypes.bfloat16),
        "v0hT": _round_f32r(np.asarray(values0, np.float32)[:T].T),
        "bhb": np.ascontiguousarray(
            np.asarray(bh, np.float32).reshape(1, H)),
        "maskRM": maskRM_cm,
    }

    Wout = np.asarray(Wout, np.float32)
    bout = np.asarray(bout, np.float32)
    in_maps = []
    for c in range(NCORES):
        wsh = Wout[:, c * VSH:(c + 1) * VSH]           # [H, VSH]
        wt = np.zeros((128, 4, VCH * 128), np.float32)
        for hc in range(4):
            wt[:, hc, :VSH] = wsh[hc * 128:(hc + 1) * 128, :]
        bt = np.zeros((128, VCH), np.float32)
        bsh = bout[c * VSH:(c + 1) * VSH]
        for vc in range(VCH):
            vsz = 128 if vc < VCH - 1 else VLAST
            bt[:vsz, vc] = bsh[vc * 128:vc * 128 + vsz]
        br = np.zeros((1, VCH * 128), np.float32)
        br[0, :VSH] = bsh
        in_maps.append({**base,
                        "woutc": wt.astype(ml_dtypes.bfloat16),
                        "boutc": bt,
                        "boutr": br.astype(ml_dtypes.bfloat16)})
    return in_maps


def run_on_device(in_maps, trace=False):
    from concourse import bass_utils
    nc = _get_program()
    return bass_utils.run_bass_kernel_spmd(
        nc, in_maps, core_ids=list(range(NCORES)), trace=trace)


def kernel(tokens, emb, Wq, Wk, Wv, Wh, bh, Wout, bout, keys0, values0, k):
    assert int(k) == K
    in_maps = _host_prep(tokens, emb, Wq, Wk, Wv, Wh, bh, Wout, bout,
                         keys0, values0)
    res = run_on_device(in_maps)
    parts = [np.asarray(res.results[c]["out"], dtype=np.float32)
             for c in range(NCORES)]                             # each [VSH, TB]
    logitsT = np.concatenate(parts, axis=0)                      # [V, TB]
    return np.ascontiguousarray(logitsT.T).reshape(T, B, V)